# revision 11
# baseline (speedup 1.0000x reference)
"""GCN message-passing layer (copy_src -> segment_sum -> dual degree norm)
on 8 Trainium2 NeuronCores.

Strategy (dst-sharded message passing):
  Host side (sharding/metadata only):
    - node_f = concat(u_f, v_f) * out_norm[src-side], cast to bf16.
      in-degree norm is folded into the per-edge weight w[e] = in_norm[dst[e]].
    - Edges bucketed by (core = dst range of 12500, block = 256-dst tile,
      window = 20000-src range so gather indices fit int16), padded to
      128-edge chunks with -1 indices (skipped by the DMA engine).
  Device side (per core, one static SPMD program):
    - gpsimd: dma_gather of the 256B bf16 source-feature rows, one call per
      (block, window) bucket, round-robined over the 4 SWDGE queues so
      descriptor generation runs on all four Q7 core pairs.
    - ACT (scalar engine): weighted one-hot S[e, slot] built in two
      activation passes (t = Square(iota - slot); S = Relu(w - w*t), exact
      for integer iota). Runs on ACT because DVE ops stall on SBUF-port
      contention with SWDGE descriptor generation.
    - PE: psum[feat(128), slot(256)] += M[e, feat].T @ S[e, slot] in bf16,
      accumulated over a block's chunks.
    - ACT: psum -> SBUF eviction per block; SP: output DMA.
  Host: transpose/concat the per-core [128 feat, 12544 slot] outputs.
"""

import math
from contextlib import ExitStack
from dataclasses import dataclass, field

import numpy as np

P = 128  # SBUF partitions / chunk size (edges per matmul)


def cdiv(a, b):
    return -(-a // b)


@dataclass(frozen=True)
class Cfg:
    n_nodes: int = 100000
    d: int = 128
    n_cores: int = 8
    blk: int = 256      # dst nodes per psum block (matmul N dim)
    win: int = 20000    # src window rows (must be < 32768 for int16 idxs)
    cpb: int = 8        # chunks per (block, window) bucket (set from data)
    nb_m: int = 8       # gather-destination (M tile) buffers
    nb_s: int = 8       # one-hot (S tile) buffers
    nsv: int = 3        # of the nb_s buffers, how many DVE builds (rest ACT)

    @property
    def dpc(self):  # dst nodes per core
        return self.n_nodes // self.n_cores

    @property
    def nblk(self):  # blocks per core
        return cdiv(self.dpc, self.blk)

    @property
    def n_win(self):
        return cdiv(self.n_nodes, self.win)

    @property
    def ncalls(self):  # gather calls per core (= buckets per core)
        return self.nblk * self.n_win

    @property
    def nchunks(self):
        return self.ncalls * self.cpb

    @property
    def spc(self):  # padded idx slots per call
        return self.cpb * P


def prep_host(u_f, v_f, src, dst, cfg: Cfg | None = None):
    """Bucket/pad edges; returns (cfg, per-core input maps)."""
    import ml_dtypes

    u_f = np.asarray(u_f, dtype=np.float32)
    v_f = np.asarray(v_f, dtype=np.float32)
    src = np.asarray(src).astype(np.int64)
    dst = np.asarray(dst).astype(np.int64)
    base = cfg or Cfg()
    N, NC = base.n_nodes, base.n_cores
    E = src.shape[0]

    node_f = np.concatenate([u_f, v_f], axis=0)
    assert node_f.shape == (N, base.d)

    deg_out = np.bincount(src, minlength=N).astype(np.float32)
    deg_in = np.bincount(dst, minlength=N).astype(np.float32)
    out_norm = np.power(np.clip(deg_out, 1.0, None), np.float32(-0.5))
    in_norm = np.power(np.clip(deg_in, 1.0, None), np.float32(-0.5))
    # out-degree norm folds into the node features; in-degree norm into the
    # per-edge one-hot weight.
    node_f = np.ascontiguousarray(
        (node_f * out_norm[:, None]).astype(ml_dtypes.bfloat16)
    )
    w_edge = in_norm[dst].astype(np.float32)

    core = dst // base.dpc
    dst_loc = dst % base.dpc
    blk_id = dst_loc // base.blk
    slot = (dst_loc % base.blk).astype(np.float32)
    win_id = src // base.win
    idx16 = (src % base.win).astype(np.int16)

    nblk, W = base.nblk, base.n_win
    bucket = (core * nblk + blk_id) * W + win_id
    nbuckets = NC * nblk * W
    counts = np.bincount(bucket, minlength=nbuckets)
    # The SWDGE descriptor ring holds ~1024 descriptors; one gather call
    # per bucket requires every bucket to stay under that.
    assert counts.max() <= 1024, (
        f"bucket overflow: {counts.max()} edges > 1024; reduce cfg.win"
    )
    cpb = max(1, cdiv(int(counts.max()), P))
    cfg = Cfg(
        n_nodes=base.n_nodes, d=base.d, n_cores=base.n_cores, blk=base.blk,
        win=base.win, cpb=cpb, nb_m=base.nb_m, nb_s=base.nb_s, nsv=base.nsv,
    )
    S = cfg.spc

    order = np.argsort(bucket, kind="stable")
    starts = np.zeros(nbuckets + 1, np.int64)
    np.cumsum(counts, out=starts[1:])
    offs = np.arange(E, dtype=np.int64) - starts[bucket[order]]
    pos = bucket[order] * S + offs

    idx_stream = np.full(nbuckets * S, -1, np.int16)
    slot_stream = np.zeros(nbuckets * S, np.float32)
    w_stream = np.zeros(nbuckets * S, np.float32)
    idx_stream[pos] = idx16[order]
    slot_stream[pos] = slot[order]
    w_stream[pos] = w_edge[order]

    cnts = counts.astype(np.int32)
    empty = cnts == 0
    if empty.any():
        # dma_gather needs >= 1 valid idx per call; gather row 0 with w=0.
        idx_stream[np.nonzero(empty)[0] * S] = 0
        cnts[empty] = 1

    per_core = cfg.ncalls * S
    in_maps = []
    for c in range(NC):
        seg = slice(c * per_core, (c + 1) * per_core)
        xi = idx_stream[seg].reshape(cfg.ncalls, S // 16, 16)
        xi = np.ascontiguousarray(
            np.tile(xi.transpose(2, 0, 1).reshape(16, -1), (8, 1))
        )
        # slots negated: ACT pass 1 computes Square(iota + bias), bias=-slot.
        sl = np.ascontiguousarray(-slot_stream[seg].reshape(-1, P).T)
        wv = w_stream[seg].reshape(-1, P).T
        wpos = np.ascontiguousarray(wv)
        wneg = np.ascontiguousarray(-wv)
        cn = np.ascontiguousarray(
            cnts[c * cfg.ncalls:(c + 1) * cfg.ncalls].reshape(1, -1)
        )
        in_maps.append(
            {"nf": node_f, "idx": xi, "slots": sl, "wpos": wpos,
             "wneg": wneg, "ncnt": cn}
        )
    return cfg, in_maps


def build_nc(cfg: Cfg):
    import concourse.bacc as bacc
    import concourse.mybir as mybir
    from concourse.library_config import mlp

    f32 = mybir.dt.float32
    bf16 = mybir.dt.bfloat16
    AF = mybir.ActivationFunctionType
    D, W, cpb, nblk = cfg.d, cfg.n_win, cfg.cpb, cfg.nblk
    ncalls, nchunks = cfg.ncalls, cfg.nchunks
    idx_cols = ncalls * cfg.spc // 16

    nc = bacc.Bacc("TRN2", target_bir_lowering=False, num_swdge_queues=4)

    nf = nc.dram_tensor("nf", [cfg.n_nodes, D], bf16, kind="ExternalInput")
    idx_d = nc.dram_tensor("idx", [P, idx_cols], mybir.dt.int16, kind="ExternalInput")
    slots_d = nc.dram_tensor("slots", [P, nchunks], f32, kind="ExternalInput")
    wpos_d = nc.dram_tensor("wpos", [P, nchunks], f32, kind="ExternalInput")
    wneg_d = nc.dram_tensor("wneg", [P, nchunks], f32, kind="ExternalInput")
    cnt_d = nc.dram_tensor("ncnt", [1, ncalls], mybir.dt.int32, kind="ExternalInput")
    out_d = nc.dram_tensor("out", [P, nblk * cfg.blk], f32, kind="ExternalOutput")

    with ExitStack() as ctx:
        ec = ctx.enter_context
        # S/iota tiles get an odd free dim (blk+1) so DVE tensor_scalar
        # auto-detects a 1-port perf mode: 2-port DVE modes interlock with
        # SWDGE descriptor generation on the shared POOL SBUF slot and stall
        # for the remainder of the in-flight gather call.
        sfd = cfg.blk + 1
        idx_sb = ec(nc.sbuf_tensor("idx_sb", [P, idx_cols], mybir.dt.int16))
        slots_sb = ec(nc.sbuf_tensor("slots_sb", [P, nchunks], f32))
        wpos_sb = ec(nc.sbuf_tensor("wpos_sb", [P, nchunks], f32))
        wneg_sb = ec(nc.sbuf_tensor("wneg_sb", [P, nchunks], f32))
        cnt_sb = ec(nc.sbuf_tensor("cnt_sb", [1, ncalls], mybir.dt.int32))
        iota_sb = ec(nc.sbuf_tensor("iota_sb", [P, sfd], bf16))
        niota_sb = ec(nc.sbuf_tensor("niota_sb", [P, sfd], bf16))
        m_sbs = [ec(nc.sbuf_tensor(f"m{j}", [P, cpb, D], bf16)) for j in range(cfg.nb_m)]
        s_sbs = [ec(nc.sbuf_tensor(f"s{j}", [P, sfd], bf16)) for j in range(cfg.nb_s)]
        t_sbs = [ec(nc.sbuf_tensor(f"t{j}", [P, sfd], bf16)) for j in range(2)]
        obufs = [ec(nc.sbuf_tensor(f"ob{j}", [P, cfg.blk], f32)) for j in range(2)]
        psums = [ec(nc.psum_tensor(f"ps{j}", [P, cfg.blk], f32)) for j in range(2)]

        io = ec(nc.semaphore("io"))
        init = ec(nc.semaphore("init"))
        gsems = [ec(nc.semaphore(f"gat{j}")) for j in range(cfg.nb_m)]
        sv = ec(nc.semaphore("sv"))    # DVE-built S chunks
        sa = ec(nc.semaphore("sa"))    # ACT-built S chunks
        pe = ec(nc.semaphore("pe"))
        ev = ec(nc.semaphore("ev"))
        osems = [ec(nc.semaphore(f"odma{j}")) for j in range(2)]

        nsv, nsa = cfg.nsv, cfg.nb_s - cfg.nsv

        def builder(t):
            """(engine, count-on-that-engine's-sem when chunk t is built)."""
            j = t % cfg.nb_s
            if j < nsv:
                return "v", (t // cfg.nb_s) * nsv + j + 1
            return "a", (t // cfg.nb_s) * nsa + (j - nsv) + 1

        with nc.Block() as block:

            @block.sync
            def _(sync):
                sync.dma_start(idx_sb[:], idx_d[:]).then_inc(io, 16)
                sync.dma_start(slots_sb[:], slots_d[:]).then_inc(io, 16)
                sync.dma_start(wpos_sb[:], wpos_d[:]).then_inc(io, 16)
                sync.dma_start(wneg_sb[:], wneg_d[:]).then_inc(io, 16)
                sync.dma_start(cnt_sb[:], cnt_d[:]).then_inc(io, 16)
                for b in range(nblk):
                    sync.wait_ge(ev, b + 1)
                    sync.dma_start(
                        out_d[:, b * cfg.blk:(b + 1) * cfg.blk], obufs[b % 2][:]
                    ).then_inc(osems[b % 2], 16)
                sync.wait_ge(osems[0], 16 * cdiv(nblk, 2))
                if nblk > 1:
                    sync.wait_ge(osems[1], 16 * (nblk // 2))

            @block.gpsimd
            def _(g):
                g.iota(
                    iota_sb[:], [[1, sfd]], channel_multiplier=0,
                    allow_small_or_imprecise_dtypes=True,
                ).then_inc(init, 1)
                g.iota(
                    niota_sb[:], [[-1, sfd]], channel_multiplier=0,
                    allow_small_or_imprecise_dtypes=True,
                ).then_inc(init, 1)
                for j in range(cfg.nb_m):
                    g.memset(m_sbs[j][:], 0).then_inc(init, 1)
                g.load_library(mlp)
                g.wait_ge(init, 2 + cfg.nb_m)
                g.wait_ge(io, 80)
                with g.register("cnt") as cnt:
                    for k in range(ncalls):
                        w = k % W
                        if k >= cfg.nb_m:
                            g.wait_ge(pe, (k - cfg.nb_m + 1) * cpb)
                        g.reg_load(cnt, cnt_sb[0:1, k:k + 1])
                        rows = min(cfg.win, cfg.n_nodes - w * cfg.win)
                        g.dma_gather(
                            m_sbs[k % cfg.nb_m][:],
                            nf[w * cfg.win: w * cfg.win + rows, :],
                            idx_sb[:, k * cpb * 8:(k + 1) * cpb * 8],
                            cfg.spc,
                            cnt,
                            D,
                            queue_num=k % 4,
                        ).then_inc(gsems[k % cfg.nb_m], 16)

            @block.vector
            def _(v):
                v.wait_ge(io, 80)
                v.wait_ge(init, 2)
                for t in range(nchunks):
                    if t % cfg.nb_s >= nsv:
                        continue
                    if t >= cfg.nb_s:
                        v.wait_ge(pe, t - cfg.nb_s + 1)
                    v.tensor_scalar(
                        out=s_sbs[t % cfg.nb_s][:],
                        in0=niota_sb[:],
                        scalar1=slots_sb[:, t:t + 1],
                        scalar2=wpos_sb[:, t:t + 1],
                        op0=mybir.AluOpType.is_equal,
                        op1=mybir.AluOpType.mult,
                    ).then_inc(sv, 1)

            @block.scalar
            def _(a):
                a.wait_ge(io, 80)
                a.wait_ge(init, 1)
                # evict(b) is emitted after the first ACT S-build at or past
                # (b+1)*W*cpb-1+nb_s so the PE has pre-built chunks to chew
                # on during the eviction.
                triggers = [
                    (min((b + 1) * W * cpb - 1 + cfg.nb_s, nchunks - 1), b)
                    for b in range(nblk)
                ]
                triggers.reverse()  # pop from the end in ascending order

                def emit_evict(b):
                    a.wait_ge(pe, (b + 1) * W * cpb)
                    if b >= 2:
                        a.wait_ge(osems[b % 2], 16 * (b // 2))
                    a.activation(
                        obufs[b % 2][:], psums[b % 2][:], AF.Copy,
                    ).then_inc(ev, 1)

                for t in range(nchunks):
                    if t % cfg.nb_s < nsv:
                        continue
                    if t >= cfg.nb_s:
                        a.wait_ge(pe, t - cfg.nb_s + 1)
                    a.activation(
                        t_sbs[t % 2][:], iota_sb[:], AF.Square,
                        bias=slots_sb[:, t:t + 1],
                    )
                    a.activation(
                        s_sbs[t % cfg.nb_s][:], t_sbs[t % 2][:], AF.Relu,
                        bias=wpos_sb[:, t:t + 1], scale=wneg_sb[:, t:t + 1],
                    ).then_inc(sa, 1)
                    while triggers and triggers[-1][0] <= t:
                        emit_evict(triggers.pop()[1])
                while triggers:
                    emit_evict(triggers.pop()[1])

            @block.tensor
            def _(te):
                t = 0
                for b in range(nblk):
                    for w in range(W):
                        k = b * W + w
                        for i in range(cpb):
                            if i == 0:
                                te.wait_ge(
                                    gsems[k % cfg.nb_m],
                                    16 * (k // cfg.nb_m + 1),
                                )
                            eng, cnt_needed = builder(t)
                            te.wait_ge(sv if eng == "v" else sa, cnt_needed)
                            start = (w == 0 and i == 0)
                            stop = (w == W - 1 and i == cpb - 1)
                            if start and b >= 2:
                                te.wait_ge(ev, b - 1)
                            te.matmul(
                                psums[b % 2][:],
                                m_sbs[k % cfg.nb_m][:, i, :],
                                s_sbs[t % cfg.nb_s][:, 0:cfg.blk],
                                start=start,
                                stop=stop,
                            ).then_inc(pe, 1)
                            t += 1

    nc.compile()
    return nc


def unshard(cfg: Cfg, results):
    out = np.empty((cfg.n_nodes, cfg.d), np.float32)
    for c in range(cfg.n_cores):
        o = results[c]["out"]
        out[c * cfg.dpc:(c + 1) * cfg.dpc, :] = o[:, :cfg.dpc].T
    return out


def run(inputs, trace=False, **spmd_kwargs):
    from concourse.bass_utils import run_bass_kernel_spmd

    cfg, in_maps = prep_host(
        inputs["u_f"], inputs["v_f"], inputs["src"], inputs["dst"]
    )
    nc = build_nc(cfg)
    res = run_bass_kernel_spmd(
        nc, in_maps, core_ids=list(range(cfg.n_cores)), trace=trace,
        **spmd_kwargs,
    )
    return unshard(cfg, res.results), res


def kernel(**inputs):
    return run(inputs)[0]


# revision 19
# speedup vs baseline: 1.2041x; 1.2041x over previous
"""GCN message-passing layer (copy_src -> segment_sum -> dual degree norm)
on 8 Trainium2 NeuronCores.

Strategy (dst-sharded message passing):
  Host side (sharding/metadata only):
    - node_f = concat(u_f, v_f) * out_norm[src-side], cast to bf16.
      in-degree norm is folded into the per-edge weight w[e] = in_norm[dst[e]].
    - Edges bucketed by (core = dst range of 12500, block = 256-dst tile,
      window = 20000-src range so gather indices fit int16), padded to
      128-edge chunks with -1 indices (skipped by the DMA engine).
  Device side (per core, one static SPMD program):
    - gpsimd: dma_gather of the 256B bf16 source-feature rows, one call per
      (block, window) bucket, round-robined over the 4 SWDGE queues so
      descriptor generation runs on all four Q7 core pairs.
    - ACT (scalar engine): weighted one-hot S[e, slot] built in two
      activation passes (t = Square(iota - slot); S = Relu(w - w*t), exact
      for integer iota). Runs on ACT because DVE ops stall on SBUF-port
      contention with SWDGE descriptor generation.
    - PE: psum[feat(128), slot(256)] += M[e, feat].T @ S[e, slot] in bf16,
      accumulated over a block's chunks.
    - ACT: psum -> SBUF eviction per block; SP: output DMA.
  Host: transpose/concat the per-core [128 feat, 12544 slot] outputs.
"""

import math
from contextlib import ExitStack
from dataclasses import dataclass, field

import numpy as np

P = 128  # SBUF partitions / chunk size (edges per matmul)


def cdiv(a, b):
    return -(-a // b)


@dataclass(frozen=True)
class Cfg:
    n_nodes: int = 100000
    d: int = 128
    n_cores: int = 8
    blk: int = 256      # dst nodes per psum block (matmul N dim)
    win: int = 20000    # src window rows (must be < 32768 for int16 idxs)
    cpb: int = 8        # chunks per (block, window) bucket (set from data)
    nb_m: int = 8       # gather-destination (M tile) buffers
    nb_s: int = 8       # one-hot (S tile) buffers
    nsv: int = 4        # of the nb_s buffers, how many DVE builds (rest ACT)
    ck: tuple = ()      # per-call chunk counts (max across cores; from data)

    @property
    def dpc(self):  # dst nodes per core
        return self.n_nodes // self.n_cores

    @property
    def nblk(self):  # blocks per core
        return cdiv(self.dpc, self.blk)

    @property
    def n_win(self):
        return cdiv(self.n_nodes, self.win)

    @property
    def ncalls(self):  # gather calls per core (= buckets per core)
        return self.nblk * self.n_win

    @property
    def nchunks(self):  # compact (unpadded) chunk count
        return sum(self.ck) if self.ck else self.ncalls * self.cpb

    @property
    def prefix(self):  # chunk index of each call's first chunk
        p = [0]
        for c in self.ck:
            p.append(p[-1] + c)
        return p

    @property
    def spc(self):  # padded idx slots per call
        return self.cpb * P


def prep_host(u_f, v_f, src, dst, cfg: Cfg | None = None):
    """Bucket/pad edges; returns (cfg, per-core input maps)."""
    import ml_dtypes

    u_f = np.asarray(u_f, dtype=np.float32)
    v_f = np.asarray(v_f, dtype=np.float32)
    src = np.asarray(src).astype(np.int64)
    dst = np.asarray(dst).astype(np.int64)
    base = cfg or Cfg()
    N, NC = base.n_nodes, base.n_cores
    E = src.shape[0]

    node_f = np.concatenate([u_f, v_f], axis=0)
    assert node_f.shape == (N, base.d)

    deg_out = np.bincount(src, minlength=N).astype(np.float32)
    deg_in = np.bincount(dst, minlength=N).astype(np.float32)
    out_norm = np.power(np.clip(deg_out, 1.0, None), np.float32(-0.5))
    in_norm = np.power(np.clip(deg_in, 1.0, None), np.float32(-0.5))
    # out-degree norm folds into the node features; in-degree norm into the
    # per-edge one-hot weight.
    node_f = np.ascontiguousarray(
        (node_f * out_norm[:, None]).astype(ml_dtypes.bfloat16)
    )
    w_edge = in_norm[dst].astype(np.float32)

    core = dst // base.dpc
    dst_loc = dst % base.dpc
    blk_id = dst_loc // base.blk
    slot = (dst_loc % base.blk).astype(np.float32)
    win_id = src // base.win
    idx16 = (src % base.win).astype(np.int16)

    nblk, W = base.nblk, base.n_win
    bucket = (core * nblk + blk_id) * W + win_id
    nbuckets = NC * nblk * W
    counts = np.bincount(bucket, minlength=nbuckets)
    # The SWDGE descriptor ring holds ~1024 descriptors; one gather call
    # per bucket requires every bucket to stay under that.
    assert counts.max() <= 1024, (
        f"bucket overflow: {counts.max()} edges > 1024; reduce cfg.win"
    )
    cpb = max(1, cdiv(int(counts.max()), P))
    ncalls = base.ncalls
    # Per-call chunk counts: max across cores so one SPMD program fits all.
    ck = np.maximum(
        1, cdiv(counts.reshape(NC, ncalls), P).max(axis=0)
    ).astype(np.int64)
    cfg = Cfg(
        n_nodes=base.n_nodes, d=base.d, n_cores=base.n_cores, blk=base.blk,
        win=base.win, cpb=cpb, nb_m=base.nb_m, nb_s=base.nb_s, nsv=base.nsv,
        ck=tuple(int(x) for x in ck),
    )
    S = cfg.spc
    nch = cfg.nchunks
    prefix = np.asarray(cfg.prefix[:-1], np.int64)

    order = np.argsort(bucket, kind="stable")
    starts = np.zeros(nbuckets + 1, np.int64)
    np.cumsum(counts, out=starts[1:])
    offs = np.arange(E, dtype=np.int64) - starts[bucket[order]]
    bo = bucket[order]
    pos = bo * S + offs  # padded layout for the gather idx stream
    # compact layout for slot/weight streams: call k's chunks start at
    # prefix[k] regardless of core (ck is the cross-core max).
    k_loc = bo % ncalls
    c_of = bo // ncalls
    pos_sw = (c_of * nch + prefix[k_loc]) * P + offs

    idx_stream = np.full(nbuckets * S, -1, np.int16)
    slot_stream = np.zeros(NC * nch * P, np.float32)
    w_stream = np.zeros(NC * nch * P, np.float32)
    idx_stream[pos] = idx16[order]
    slot_stream[pos_sw] = slot[order]
    w_stream[pos_sw] = w_edge[order]

    cnts = counts.astype(np.int32)
    empty = cnts == 0
    if empty.any():
        # dma_gather needs >= 1 valid idx per call; gather row 0 with w=0.
        idx_stream[np.nonzero(empty)[0] * S] = 0
        cnts[empty] = 1

    per_core = cfg.ncalls * S
    in_maps = []
    for c in range(NC):
        seg = slice(c * per_core, (c + 1) * per_core)
        xi = idx_stream[seg].reshape(cfg.ncalls, S // 16, 16)
        xi = np.ascontiguousarray(
            np.tile(xi.transpose(2, 0, 1).reshape(16, -1), (8, 1))
        )
        seg_sw = slice(c * nch * P, (c + 1) * nch * P)
        # slots negated: ACT pass 1 computes Square(iota + bias), bias=-slot.
        sl = np.ascontiguousarray(-slot_stream[seg_sw].reshape(-1, P).T)
        wv = w_stream[seg_sw].reshape(-1, P).T
        wpos = np.ascontiguousarray(wv)
        wneg = np.ascontiguousarray(-wv)
        cn = np.ascontiguousarray(
            cnts[c * cfg.ncalls:(c + 1) * cfg.ncalls].reshape(1, -1)
        )
        in_maps.append(
            {"nf": node_f, "idx": xi, "slots": sl, "wpos": wpos,
             "wneg": wneg, "ncnt": cn}
        )
    return cfg, in_maps


def build_nc(cfg: Cfg):
    import concourse.bacc as bacc
    import concourse.mybir as mybir
    from concourse.library_config import mlp

    f32 = mybir.dt.float32
    bf16 = mybir.dt.bfloat16
    AF = mybir.ActivationFunctionType
    D, W, cpb, nblk = cfg.d, cfg.n_win, cfg.cpb, cfg.nblk
    ncalls, nchunks = cfg.ncalls, cfg.nchunks
    ck = cfg.ck or (cpb,) * ncalls
    prefix = cfg.prefix if cfg.ck else [cpb * k for k in range(ncalls + 1)]
    idx_cols = ncalls * cfg.spc // 16

    nc = bacc.Bacc("TRN2", target_bir_lowering=False, num_swdge_queues=4)

    nf = nc.dram_tensor("nf", [cfg.n_nodes, D], bf16, kind="ExternalInput")
    idx_d = nc.dram_tensor("idx", [P, idx_cols], mybir.dt.int16, kind="ExternalInput")
    slots_d = nc.dram_tensor("slots", [P, nchunks], f32, kind="ExternalInput")
    wpos_d = nc.dram_tensor("wpos", [P, nchunks], f32, kind="ExternalInput")
    wneg_d = nc.dram_tensor("wneg", [P, nchunks], f32, kind="ExternalInput")
    cnt_d = nc.dram_tensor("ncnt", [1, ncalls], mybir.dt.int32, kind="ExternalInput")
    out_d = nc.dram_tensor("out", [P, nblk * cfg.blk], f32, kind="ExternalOutput")

    with ExitStack() as ctx:
        ec = ctx.enter_context
        # S/iota tiles get an odd free dim (blk+1) so DVE tensor_scalar
        # auto-detects a 1-port perf mode: 2-port DVE modes interlock with
        # SWDGE descriptor generation on the shared POOL SBUF slot and stall
        # for the remainder of the in-flight gather call.
        sfd = cfg.blk + 1
        idx_sb = ec(nc.sbuf_tensor("idx_sb", [P, idx_cols], mybir.dt.int16))
        slots_sb = ec(nc.sbuf_tensor("slots_sb", [P, nchunks], f32))
        wpos_sb = ec(nc.sbuf_tensor("wpos_sb", [P, nchunks], f32))
        wneg_sb = ec(nc.sbuf_tensor("wneg_sb", [P, nchunks], f32))
        cnt_sb = ec(nc.sbuf_tensor("cnt_sb", [1, ncalls], mybir.dt.int32))
        iota_sb = ec(nc.sbuf_tensor("iota_sb", [P, sfd], bf16))
        niota_sb = ec(nc.sbuf_tensor("niota_sb", [P, sfd], bf16))
        m_sbs = [ec(nc.sbuf_tensor(f"m{j}", [P, cpb, D], bf16)) for j in range(cfg.nb_m)]
        s_sbs = [ec(nc.sbuf_tensor(f"s{j}", [P, sfd], bf16)) for j in range(cfg.nb_s)]
        t_sbs = [ec(nc.sbuf_tensor(f"t{j}", [P, sfd], bf16)) for j in range(2)]
        obufs = [ec(nc.sbuf_tensor(f"ob{j}", [P, cfg.blk], f32)) for j in range(2)]
        psums = [ec(nc.psum_tensor(f"ps{j}", [P, cfg.blk], f32)) for j in range(2)]

        io = ec(nc.semaphore("io"))
        init = ec(nc.semaphore("init"))
        gsems = [ec(nc.semaphore(f"gat{j}")) for j in range(cfg.nb_m)]
        sv = ec(nc.semaphore("sv"))    # DVE-built S chunks
        sa = ec(nc.semaphore("sa"))    # ACT-built S chunks
        pe = ec(nc.semaphore("pe"))
        ev = ec(nc.semaphore("ev"))
        osems = [ec(nc.semaphore(f"odma{j}")) for j in range(2)]

        nsv, nsa = cfg.nsv, cfg.nb_s - cfg.nsv

        def builder(t):
            """(engine, count-on-that-engine's-sem when chunk t is built)."""
            j = t % cfg.nb_s
            if j < nsv:
                return "v", (t // cfg.nb_s) * nsv + j + 1
            return "a", (t // cfg.nb_s) * nsa + (j - nsv) + 1

        with nc.Block() as block:

            @block.sync
            def _(sync):
                sync.dma_start(idx_sb[:], idx_d[:]).then_inc(io, 16)
                sync.dma_start(slots_sb[:], slots_d[:]).then_inc(io, 16)
                sync.dma_start(wpos_sb[:], wpos_d[:]).then_inc(io, 16)
                sync.dma_start(wneg_sb[:], wneg_d[:]).then_inc(io, 16)
                sync.dma_start(cnt_sb[:], cnt_d[:]).then_inc(io, 16)
                for b in range(nblk):
                    sync.wait_ge(ev, b + 1)
                    sync.dma_start(
                        out_d[:, b * cfg.blk:(b + 1) * cfg.blk], obufs[b % 2][:]
                    ).then_inc(osems[b % 2], 16)
                sync.wait_ge(osems[0], 16 * cdiv(nblk, 2))
                if nblk > 1:
                    sync.wait_ge(osems[1], 16 * (nblk // 2))

            @block.gpsimd
            def _(g):
                g.iota(
                    iota_sb[:], [[1, sfd]], channel_multiplier=0,
                    allow_small_or_imprecise_dtypes=True,
                ).then_inc(init, 1)
                g.iota(
                    niota_sb[:], [[-1, sfd]], channel_multiplier=0,
                    allow_small_or_imprecise_dtypes=True,
                ).then_inc(init, 1)
                for j in range(cfg.nb_m):
                    g.memset(m_sbs[j][:], 0).then_inc(init, 1)
                g.load_library(mlp)
                g.wait_ge(init, 2 + cfg.nb_m)
                g.wait_ge(io, 80)
                with g.register("cnt") as cnt:
                    for k in range(ncalls):
                        w = k % W
                        if k >= cfg.nb_m:
                            g.wait_ge(pe, prefix[k - cfg.nb_m + 1])
                        g.reg_load(cnt, cnt_sb[0:1, k:k + 1])
                        rows = min(cfg.win, cfg.n_nodes - w * cfg.win)
                        g.dma_gather(
                            m_sbs[k % cfg.nb_m][:],
                            nf[w * cfg.win: w * cfg.win + rows, :],
                            idx_sb[:, k * cpb * 8:(k + 1) * cpb * 8],
                            cfg.spc,
                            cnt,
                            D,
                            queue_num=k % 4,
                        ).then_inc(gsems[k % cfg.nb_m], 16)

            @block.vector
            def _(v):
                v.wait_ge(io, 80)
                v.wait_ge(init, 2)
                for t in range(nchunks):
                    if t % cfg.nb_s >= nsv:
                        continue
                    if t >= cfg.nb_s:
                        v.wait_ge(pe, t - cfg.nb_s + 1)
                    v.tensor_scalar(
                        out=s_sbs[t % cfg.nb_s][:],
                        in0=niota_sb[:],
                        scalar1=slots_sb[:, t:t + 1],
                        scalar2=wpos_sb[:, t:t + 1],
                        op0=mybir.AluOpType.is_equal,
                        op1=mybir.AluOpType.mult,
                    ).then_inc(sv, 1)

            @block.scalar
            def _(a):
                a.wait_ge(io, 80)
                a.wait_ge(init, 1)
                # evict(b) is emitted after the first ACT S-build at or past
                # the block's last chunk + nb_s lookahead so the PE has
                # pre-built chunks to chew on during the eviction.
                triggers = [
                    (min(prefix[(b + 1) * W] - 1 + cfg.nb_s, nchunks - 1), b)
                    for b in range(nblk)
                ]
                triggers.reverse()  # pop from the end in ascending order

                def emit_evict(b):
                    a.wait_ge(pe, prefix[(b + 1) * W])
                    if b >= 2:
                        a.wait_ge(osems[b % 2], 16 * (b // 2))
                    a.activation(
                        obufs[b % 2][:], psums[b % 2][:], AF.Copy,
                    ).then_inc(ev, 1)

                for t in range(nchunks):
                    if t % cfg.nb_s < nsv:
                        continue
                    if t >= cfg.nb_s:
                        a.wait_ge(pe, t - cfg.nb_s + 1)
                    a.activation(
                        t_sbs[t % 2][:], iota_sb[:], AF.Square,
                        bias=slots_sb[:, t:t + 1],
                    )
                    a.activation(
                        s_sbs[t % cfg.nb_s][:], t_sbs[t % 2][:], AF.Relu,
                        bias=wpos_sb[:, t:t + 1], scale=wneg_sb[:, t:t + 1],
                    ).then_inc(sa, 1)
                    while triggers and triggers[-1][0] <= t:
                        emit_evict(triggers.pop()[1])
                while triggers:
                    emit_evict(triggers.pop()[1])

            @block.tensor
            def _(te):
                t = 0
                for b in range(nblk):
                    for w in range(W):
                        k = b * W + w
                        for i in range(ck[k]):
                            if i == 0:
                                te.wait_ge(
                                    gsems[k % cfg.nb_m],
                                    16 * (k // cfg.nb_m + 1),
                                )
                            eng, cnt_needed = builder(t)
                            te.wait_ge(sv if eng == "v" else sa, cnt_needed)
                            start = (w == 0 and i == 0)
                            stop = (w == W - 1 and i == ck[k] - 1)
                            if start and b >= 2:
                                te.wait_ge(ev, b - 1)
                            te.matmul(
                                psums[b % 2][:],
                                m_sbs[k % cfg.nb_m][:, i, :],
                                s_sbs[t % cfg.nb_s][:, 0:cfg.blk],
                                start=start,
                                stop=stop,
                            ).then_inc(pe, 1)
                            t += 1

    nc.compile()
    return nc


def unshard(cfg: Cfg, results):
    out = np.empty((cfg.n_nodes, cfg.d), np.float32)
    for c in range(cfg.n_cores):
        o = results[c]["out"]
        out[c * cfg.dpc:(c + 1) * cfg.dpc, :] = o[:, :cfg.dpc].T
    return out


def run(inputs, trace=False, **spmd_kwargs):
    from concourse.bass_utils import run_bass_kernel_spmd

    cfg, in_maps = prep_host(
        inputs["u_f"], inputs["v_f"], inputs["src"], inputs["dst"]
    )
    nc = build_nc(cfg)
    res = run_bass_kernel_spmd(
        nc, in_maps, core_ids=list(range(cfg.n_cores)), trace=trace,
        **spmd_kwargs,
    )
    return unshard(cfg, res.results), res


def kernel(**inputs):
    return run(inputs)[0]
